# revision 1
# baseline (speedup 1.0000x reference)
"""Multi-Head Latent Attention (MLA) Trainium2 kernel, 8-way sharded.

Sharding: 8 cores = 2 (batch) x 4 (head groups of 4 heads).
Each core handles one batch element and 4 of the 16 heads:
  - computes qc = x @ W_D_Q, c = x @ W_D_KV (replicated within batch group)
  - its 4 heads' q/k/v projections + decoupled RoPE
  - full-sequence attention for its heads (streaming; scores never hit HBM)
  - partial output projection with its W_O row-slice
Host sums the 4 partials per batch element.

All matmuls run in float32r (TF32-like single-pass PE mode: 1 cyc/row for
free-dim >= 256, ~1e-4 relative error, no cast instructions needed).

Everything is computed TRANSPOSED (feature dim on partitions):
  qc^T = W_D_Q^T x^T, q^T = W_U_Q^T qc^T, etc. Attention scores come out
as S^T (keys on partitions, queries on free axis), so:
  - softmax = plain exp (scores are O(+-6) -> no max subtraction needed)
  - row-sums via a ones-matmul accumulated alongside O^T = V^T P^T
  - no on-chip transposes anywhere; RoPE's pair-swap is a small constant
    matmul (block-diag rotation) plus two elementwise multiplies.
"""

import sys

sys.path.insert(0, "/opt/trn_rl_repo")

import numpy as np

import concourse.bacc as bacc
import concourse.mybir as mybir
import concourse.tile as tile
from concourse.bass_utils import run_bass_kernel_spmd

# Problem dims (hardcoded per contract)
D, NH, DH, DC, DCQ, DHR = 2048, 16, 128, 512, 1536, 64
B, L = 2, 2048
ROPE_THETA = 10000.0

NHG = 4                 # heads per core
DQB = NHG * DH          # 512: per-core base q/k feature dim (also v dim)
DQR = NHG * DHR         # 256: per-core rope feature dim
P = 128
CW = 512                # phase-A token chunk width (= PSUM free dim)
SCALE = DH ** -0.5

F32R = mybir.dt.float32r
F32 = mybir.dt.float32

_CACHED = {}


def _build(repeat=None):
    """Build the SPMD program. repeat=N wraps the body in a HW loop (for
    perf measurement only — amortizes host dispatch overhead)."""
    nc = bacc.Bacc("TRN2", target_bir_lowering=False, debug=False)

    # ---- DRAM I/O (per-core data; program is SPMD)
    xT = nc.dram_tensor("xT", [D, L], F32R, kind="ExternalInput")
    wdq = nc.dram_tensor("wdq", [D, DCQ], F32R, kind="ExternalInput")
    wuq = nc.dram_tensor("wuq", [DCQ, DQB], F32R, kind="ExternalInput")
    wqr = nc.dram_tensor("wqr", [DCQ, DQR], F32R, kind="ExternalInput")
    wdkv = nc.dram_tensor("wdkv", [D, DC], F32R, kind="ExternalInput")
    wuk = nc.dram_tensor("wuk", [DC, DQB], F32R, kind="ExternalInput")
    wkr = nc.dram_tensor("wkr", [D, DQR], F32R, kind="ExternalInput")
    wuv = nc.dram_tensor("wuv", [DC, DQB], F32R, kind="ExternalInput")
    wo = nc.dram_tensor("wo", [DQB, D], F32R, kind="ExternalInput")
    cosr = nc.dram_tensor("cosr", [P, L], F32, kind="ExternalInput")
    sinr = nc.dram_tensor("sinr", [P, L], F32, kind="ExternalInput")
    protT = nc.dram_tensor("protT", [P, P], F32R, kind="ExternalInput")
    onesd = nc.dram_tensor("onesd", [P, P], F32R, kind="ExternalInput")
    out = nc.dram_tensor("out", [L, D], F32, kind="ExternalOutput")

    # ---- internal DRAM spill (transposed q/k, natural v)
    qbT_d = nc.dram_tensor("qbT_d", [DQB, L], F32R)
    qrT_d = nc.dram_tensor("qrT_d", [DQR, L], F32R)
    kbT_d = nc.dram_tensor("kbT_d", [DQB, L], F32R)
    krT_d = nc.dram_tensor("krT_d", [DQR, L], F32R)
    v_d = nc.dram_tensor("v_d", [L, DQB], F32R)

    KD = D // P      # 16
    KQ = DCQ // P    # 12
    KC = DC // P     # 4

    def mm_chain(ps, w_src, rhs_tiles, nk, wpool, wtag, col):
        """ps += sum_k w_src[k-tile, col-block]^T @ rhs_tiles[k].

        All nk (128x128) weight chunks arrive in ONE batched DMA (HWDGE
        dispatch is ~625ns/instruction and serializes — it was the
        bottleneck with per-chunk DMAs)."""
        wt = wpool.tile([P, nk, P], F32R, name=f"w_{wtag}", tag="wt")
        src = w_src[:, col * P:(col + 1) * P].rearrange("(k p) j -> p k j", p=P)
        nc.sync.dma_start(out=wt[:], in_=src)
        for k in range(nk):
            nc.tensor.matmul(ps[:], wt[:, k, :], rhs_tiles[k][:],
                             start=(k == 0), stop=(k == nk - 1))

    from contextlib import nullcontext
    with tile.TileContext(nc) as tc:
        with (tc.For_i(0, repeat, 1) if repeat else nullcontext()), \
             tc.tile_pool(name="constp", bufs=1) as constp, \
             tc.tile_pool(name="otp_res", bufs=1) as otp_res:
            prot_t = constp.tile([P, P], F32R, name="prot_t", tag="prot")
            nc.sync.dma_start(out=prot_t[:], in_=protT[:, :])
            ones_t = constp.tile([P, P], F32R, name="ones_t", tag="ones")
            nc.sync.dma_start(out=ones_t[:], in_=onesd[:, :])
            oT_res = [otp_res.tile([P, L], F32R, name=f"oT{h}", tag=f"oT{h}")
                      for h in range(NHG)]

            # ================= Phase A: projections (token-chunked) =========
            with tc.tile_pool(name="xp", bufs=16) as xp, \
                 tc.tile_pool(name="qcp", bufs=12) as qcp, \
                 tc.tile_pool(name="ctp", bufs=4) as ctp, \
                 tc.tile_pool(name="wcp", bufs=3) as wcp, \
                 tc.tile_pool(name="wvp", bufs=4) as wvp, \
                 tc.tile_pool(name="rop", bufs=5) as rop, \
                 tc.tile_pool(name="evp", bufs=5) as evp, \
                 tc.tile_pool(name="evv", bufs=4) as evv, \
                 tc.tile_pool(name="rtmp", bufs=2) as rtmp, \
                 tc.tile_pool(name="csp", bufs=1) as csp, \
                 tc.tile_pool(name="psA", bufs=6, space="PSUM") as psA:

                # W_U_V k-tiles, loaded once
                wuv_ts = []
                for k in range(KC):
                    wuvt = wvp.tile([P, DQB], F32R, name="wuvt", tag="wuv")
                    nc.sync.dma_start(out=wuvt[:], in_=wuv[k * P:(k + 1) * P, :])
                    wuv_ts.append(wuvt)

                for ch in range(L // CW):
                    tsl = slice(ch * CW, (ch + 1) * CW)

                    xts = []
                    for k in range(KD):
                        xt = xp.tile([P, CW], F32R, name="xt", tag="xt")
                        nc.sync.dma_start(out=xt[:], in_=xT[k * P:(k + 1) * P, tsl])
                        xts.append(xt)

                    # qc^T slab (DCQ x CW)
                    qcts = []
                    for m in range(KQ):
                        qct = qcp.tile([P, CW], F32R, name="qct", tag="qct")
                        ps = psA.tile([P, CW], F32, name="ps_qc", tag="psa")
                        mm_chain(ps, wdq, xts, KD, wcp, "dq", m)
                        nc.any.tensor_copy(qct[:], ps[:])
                        qcts.append(qct)

                    # c^T slab (DC x CW)
                    cts = []
                    for m in range(KC):
                        ct = ctp.tile([P, CW], F32R, name="ct", tag="ct")
                        ps = psA.tile([P, CW], F32, name="ps_c", tag="psa")
                        mm_chain(ps, wdkv, xts, KD, wcp, "dkv", m)
                        nc.any.tensor_copy(ct[:], ps[:])
                        cts.append(ct)

                    # k_rope^T raw (DQR x CW) — held for RoPE below
                    krts = []
                    for m in range(DQR // P):
                        krt = rop.tile([P, CW], F32R, name="krt", tag="rop")
                        ps = psA.tile([P, CW], F32, name="ps_kr", tag="psa")
                        mm_chain(ps, wkr, xts, KD, wcp, "kr", m)
                        nc.any.tensor_copy(krt[:], ps[:])
                        krts.append(krt)
                    # (xts free after krT — released on slot reuse next chunk)

                    # k_base^T (DQB x CW) -> spill
                    for m in range(DQB // P):
                        kbt = evp.tile([P, CW], F32R, name="kbt", tag="ev")
                        ps = psA.tile([P, CW], F32, name="ps_kb", tag="psa")
                        mm_chain(ps, wuk, cts, KC, wcp, "uk", m)
                        nc.any.tensor_copy(kbt[:], ps[:])
                        nc.sync.dma_start(out=kbT_d[m * P:(m + 1) * P, tsl], in_=kbt[:])

                    # v natural (CW tokens x DQB) -> spill
                    for lt in range(CW // P):
                        vt = evv.tile([P, DQB], F32R, name="vt", tag="evv")
                        ps = psA.tile([P, DQB], F32, name="ps_v", tag="psa")
                        for k in range(KC):
                            nc.tensor.matmul(
                                ps[:], cts[k][:, lt * P:(lt + 1) * P], wuv_ts[k][:],
                                start=(k == 0), stop=(k == KC - 1))
                        nc.any.tensor_copy(vt[:], ps[:])
                        nc.sync.dma_start(
                            out=v_d[ch * CW + lt * P: ch * CW + (lt + 1) * P, :],
                            in_=vt[:])

                    # q_base^T (DQB x CW) -> spill
                    for m in range(DQB // P):
                        qbt = evp.tile([P, CW], F32R, name="qbt", tag="ev")
                        ps = psA.tile([P, CW], F32, name="ps_qb", tag="psa")
                        mm_chain(ps, wuq, qcts, KQ, wcp, "uq", m)
                        nc.any.tensor_copy(qbt[:], ps[:])
                        nc.sync.dma_start(out=qbT_d[m * P:(m + 1) * P, tsl], in_=qbt[:])

                    # q_rope^T raw (DQR x CW)
                    qrts = []
                    for m in range(DQR // P):
                        qrt = rop.tile([P, CW], F32R, name="qrt", tag="rop")
                        ps = psA.tile([P, CW], F32, name="ps_qr", tag="psa")
                        mm_chain(ps, wqr, qcts, KQ, wcp, "qr", m)
                        nc.any.tensor_copy(qrt[:], ps[:])
                        qrts.append(qrt)

                    # RoPE: final = cos (.) raw + sin (.) (Prot @ raw)
                    cos_t = csp.tile([P, CW], F32, name="cos_t", tag="cos")
                    nc.sync.dma_start(out=cos_t[:], in_=cosr[:, tsl])
                    sin_t = csp.tile([P, CW], F32, name="sin_t", tag="sin")
                    nc.sync.dma_start(out=sin_t[:], in_=sinr[:, tsl])
                    for raws, dst in ((qrts, qrT_d), (krts, krT_d)):
                        for m, raw in enumerate(raws):
                            rps = psA.tile([P, CW], F32, name="rps", tag="rps", bufs=2)
                            nc.tensor.matmul(rps[:], prot_t[:], raw[:],
                                             start=True, stop=True)
                            t1 = rtmp.tile([P, CW], F32, name="t1", tag="t1")
                            nc.any.tensor_mul(t1[:], cos_t[:], raw[:])
                            t2 = rtmp.tile([P, CW], F32, name="t2", tag="t2")
                            nc.any.tensor_mul(t2[:], sin_t[:], rps[:])
                            fin = evp.tile([P, CW], F32R, name="fin", tag="ev")
                            nc.any.tensor_add(fin[:], t1[:], t2[:])
                            nc.sync.dma_start(out=dst[m * P:(m + 1) * P, tsl],
                                              in_=fin[:])

            # ================= Phase B: attention ===========================
            LQ = 512
            with tc.tile_pool(name="khp", bufs=2) as khp, \
                 tc.tile_pool(name="vhp", bufs=2) as vhp, \
                 tc.tile_pool(name="qlq", bufs=3) as qlqp, \
                 tc.tile_pool(name="ptp", bufs=4) as ptp, \
                 tc.tile_pool(name="rcp", bufs=2) as rcp, \
                 tc.tile_pool(name="stp", bufs=3, space="PSUM") as stp, \
                 tc.tile_pool(name="otp", bufs=2, space="PSUM") as otp, \
                 tc.tile_pool(name="rsp", bufs=2, space="PSUM") as rsp:
                for h in range(NHG):
                    kb_h = khp.tile([P, L], F32R, name="kb_h", tag="kb")
                    nc.sync.dma_start(out=kb_h[:], in_=kbT_d[h * P:(h + 1) * P, :])
                    kr_h = khp.tile([DHR, L], F32R, name="kr_h", tag="kr")
                    nc.sync.dma_start(out=kr_h[:], in_=krT_d[h * DHR:(h + 1) * DHR, :])
                    # all 16 (128x128) V k-tiles for this head in one DMA
                    v_h = vhp.tile([P, L // P, P], F32R, name="v_h", tag="vh")
                    nc.sync.dma_start(
                        out=v_h[:],
                        in_=v_d[:, h * DH:(h + 1) * DH].rearrange(
                            "(lk p) j -> p lk j", p=P))
                    vts = [v_h[:, lk, :] for lk in range(L // P)]
                    for lq in range(L // LQ):
                        qsl = slice(lq * LQ, (lq + 1) * LQ)
                        qb_lq = qlqp.tile([P, LQ], F32R, name="qb_lq", tag="qb")
                        nc.sync.dma_start(out=qb_lq[:],
                                          in_=qbT_d[h * P:(h + 1) * P, qsl])
                        qr_lq = qlqp.tile([DHR, LQ], F32R, name="qr_lq", tag="qr")
                        nc.sync.dma_start(out=qr_lq[:],
                                          in_=qrT_d[h * DHR:(h + 1) * DHR, qsl])

                        ot_ps = otp.tile([P, LQ], F32, name="ot_ps", tag="ot")
                        rs_ps = rsp.tile([P, LQ], F32, name="rs_ps", tag="rs")
                        for lk in range(L // P):
                            st_ps = stp.tile([P, LQ], F32, name="st_ps", tag="st")
                            nc.tensor.matmul(
                                st_ps[:], kb_h[:, lk * P:(lk + 1) * P], qb_lq[:],
                                start=True, stop=False)
                            nc.tensor.matmul(
                                st_ps[:], kr_h[:, lk * P:(lk + 1) * P], qr_lq[:],
                                start=False, stop=True)
                            pt = ptp.tile([P, LQ], F32R, name="pt", tag="pt")
                            nc.scalar.activation(
                                pt[:], st_ps[:], mybir.ActivationFunctionType.Exp,
                                scale=SCALE)
                            nc.tensor.matmul(
                                ot_ps[:], vts[lk][:], pt[:],
                                start=(lk == 0), stop=(lk == L // P - 1))
                            nc.tensor.matmul(
                                rs_ps[:], ones_t[:], pt[:],
                                start=(lk == 0), stop=(lk == L // P - 1))
                        rec = rcp.tile([P, LQ], F32, name="rec", tag="rec")
                        nc.vector.reciprocal(rec[:], rs_ps[:])
                        nc.any.tensor_mul(oT_res[h][:, qsl], ot_ps[:], rec[:])

            # ================= Phase C: output projection ===================
            with tc.tile_pool(name="wop", bufs=4) as wop, \
                 tc.tile_pool(name="ocp", bufs=6) as ocp, \
                 tc.tile_pool(name="psC", bufs=4, space="PSUM") as psC:
                wots = []
                for k in range(NHG):
                    wot = wop.tile([P, D], F32R, name="wot", tag="wo")
                    nc.sync.dma_start(out=wot[:], in_=wo[k * P:(k + 1) * P, :])
                    wots.append(wot)
                for mt in range(L // P):
                    for nt in range(D // 512):
                        ps = psC.tile([P, 512], F32, name="ps_o", tag="psc")
                        for k in range(NHG):
                            nc.tensor.matmul(
                                ps[:], oT_res[k][:, mt * P:(mt + 1) * P],
                                wots[k][:, nt * 512:(nt + 1) * 512],
                                start=(k == 0), stop=(k == NHG - 1))
                        oc = ocp.tile([P, 512], F32, name="oc", tag="oc")
                        nc.any.tensor_copy(oc[:], ps[:])
                        nc.sync.dma_start(
                            out=out[mt * P:(mt + 1) * P, nt * 512:(nt + 1) * 512],
                            in_=oc[:])

    nc.compile()
    return nc


def _rope_tables():
    """cos/sin in transposed, 2-head-replicated layout (128 x L), plus Prot^T."""
    inv_freq = 1.0 / (ROPE_THETA ** (np.arange(0, DHR, 2, dtype=np.float32) / DHR))
    ang = np.arange(L, dtype=np.float32)[:, None] * inv_freq[None, :]  # (L, 32)
    cos64 = np.concatenate([np.cos(ang), np.cos(ang)], axis=1).T  # (64, L)
    sin64 = np.concatenate([np.sin(ang), np.sin(ang)], axis=1).T
    cosr = np.ascontiguousarray(np.tile(cos64, (2, 1)), dtype=np.float32)
    sinr = np.ascontiguousarray(np.tile(sin64, (2, 1)), dtype=np.float32)
    # rot(x) = [-x2, x1] per 64-dim head: Prot rows 0:32 = -I at cols 32:64,
    # rows 32:64 = +I at cols 0:32; block-diag over 2 heads; pass transposed.
    p64 = np.zeros((DHR, DHR), dtype=np.float32)
    half = DHR // 2
    p64[np.arange(half), np.arange(half) + half] = -1.0
    p64[np.arange(half) + half, np.arange(half)] = 1.0
    p128 = np.zeros((P, P), dtype=np.float32)
    p128[:DHR, :DHR] = p64
    p128[DHR:, DHR:] = p64
    protT = np.ascontiguousarray(p128.T)
    return cosr, sinr, protT


def kernel(x, W_D_Q, W_U_Q, W_Q_R, W_D_KV, W_U_K, W_K_R, W_U_V, W_O):
    if "nc" not in _CACHED:
        _CACHED["nc"] = _build()
    nc = _CACHED["nc"]

    cosr, sinr, protT = _rope_tables()
    f = np.float32
    xTs = [np.ascontiguousarray(np.asarray(x)[b].T, dtype=f) for b in range(B)]
    W_D_Q = np.ascontiguousarray(W_D_Q, dtype=f)
    W_D_KV = np.ascontiguousarray(W_D_KV, dtype=f)
    in_maps = []
    for c in range(8):
        b, g = c // 4, c % 4
        hb = slice(g * DQB, (g + 1) * DQB)
        hr = slice(g * DQR, (g + 1) * DQR)
        in_maps.append(dict(
            xT=xTs[b],
            wdq=W_D_Q,
            wuq=np.ascontiguousarray(np.asarray(W_U_Q)[:, hb], dtype=f),
            wqr=np.ascontiguousarray(np.asarray(W_Q_R)[:, hr], dtype=f),
            wdkv=W_D_KV,
            wuk=np.ascontiguousarray(np.asarray(W_U_K)[:, hb], dtype=f),
            wkr=np.ascontiguousarray(np.asarray(W_K_R)[:, hr], dtype=f),
            wuv=np.ascontiguousarray(np.asarray(W_U_V)[:, hb], dtype=f),
            wo=np.ascontiguousarray(np.asarray(W_O)[hb, :], dtype=f),
            cosr=cosr, sinr=sinr, protT=protT,
            onesd=np.ones((P, P), dtype=f),
        ))
    res = run_bass_kernel_spmd(nc, in_maps, core_ids=list(range(8)))
    outs = [r["out"] for r in res.results]
    full = np.stack(
        [outs[b * 4] + outs[b * 4 + 1] + outs[b * 4 + 2] + outs[b * 4 + 3]
         for b in range(B)]).astype(np.float32)
    return full

